# revision 22
# baseline (speedup 1.0000x reference)
"""Trainium2 Bass kernel for the dual-modality dense transformer block.

Problem (hardcoded shapes): B=8, L=1024, H=512, NH=8, HD=64.
  - 6 linear projections (q/k/v for img and txt streams)
  - 4 full attentions: (q_img,KV_img), (q_txt,KV_txt), (q_img,KV_txt), (q_txt,KV_img)
  - out_img/out_txt linears on the averaged contexts, concat + cat linear
  - attention pooling (nn.MultiheadAttention-style) + out_proj

Sharding: pure data-parallel over batch B=8 across the 8 NeuronCores.

Device algorithm (per core, one batch element): identical math to the
previous version (feature-major activations, transposed scores, ones-
augmented V for free softmax denominators, lag-1 DVE normalization with a
DMA-broadcast reciprocal).  What changed is the EMISSION SCHEDULE: the
Scalar engine's exp stream (320 activations x ~1us = the kernel's floor)
is kept saturated by interleaving every projection as small "filler"
chunks inside the attention windows instead of as serial blocks between
attentions:
  - head: only q_img/k_img chunk 0+1 before attention 1 starts
  - v_img is emitted between A1's first scores and first PV group
  - remaining q/k chunks and all txt projections ride A1's windows
  - out_img rides A3, out_txt/cat/in_proj(k) token-half 0 rides A4-ih1
  - in_proj(q) and the out_proj epilogue ride A5's windows
This removes the ~110us of Scalar-engine idle (phase-boundary
serialization) the previous schedule had.
"""

import numpy as np
import ml_dtypes

import concourse.bass as bass
import concourse.tile as tile
from concourse import bacc, mybir
from concourse.bass_utils import run_bass_kernel_spmd
from concourse.dve_ops import RECIP_APPROX_FAST_CONSTS, RECIPROCAL_APPROX_FAST

B, L, H, NH, HD = 8, 1024, 512, 8, 64
BF = mybir.dt.bfloat16
F32 = mybir.dt.float32
Exp = mybir.ActivationFunctionType.Exp
bf16 = ml_dtypes.bfloat16

N_CORES = 8


def _emit(tc, d):
    nc = tc.nc
    import contextlib

    ctx = contextlib.ExitStack()
    with ctx:
        const = ctx.enter_context(tc.tile_pool(name="const", bufs=1))
        acts = ctx.enter_context(tc.tile_pool(name="acts", bufs=1))
        spool = ctx.enter_context(tc.tile_pool(name="spool", bufs=2))
        opool = ctx.enter_context(tc.tile_pool(name="opool", bufs=1))
        expool = ctx.enter_context(tc.tile_pool(name="expool", bufs=2))
        small = ctx.enter_context(tc.tile_pool(name="small", bufs=2))
        dscr = ctx.enter_context(tc.tile_pool(name="dscr", bufs=4, space="DRAM"))
        pmm = ctx.enter_context(tc.tile_pool(name="pmm", bufs=2, space="PSUM"))
        pctx = ctx.enter_context(tc.tile_pool(name="pctx", bufs=2, space="PSUM"))

        # ---- constants / inputs into SBUF (critical-path order) ----
        def load(name, p_chunks, free, dt=BF):
            t = const.tile([128, p_chunks, free], dt, tag=name, name=name)
            src_r = d[name].rearrange("(c p) n -> p c n", p=128)
            for c in range(p_chunks):
                nc.sync.dma_start(out=t[:, c, :], in_=src_r[:, c, :])
            return t

        def load_act(name, tag):
            t = acts.tile([128, 4, L], BF, tag=tag, name=name)
            src_r = d[name].rearrange("(c p) n -> p c n", p=128)
            for c in range(4):
                nc.sync.dma_start(out=t[:, c, :], in_=src_r[:, c, :])
            return t

        def load2d(name, p, free, dt):
            t = const.tile([p, free], dt, tag=name, name=name)
            nc.sync.dma_start(out=t, in_=d[name])
            return t

        xt = load_act("xT", "xT")
        w_qim = load("w_qim", 4, H)
        b_qim = load2d("b_qim", 128, 4, F32)
        w_kim = load("w_kim", 4, H)
        b_kim = load2d("b_kim", 128, 4, F32)
        w_vim = load("w_vim", 4, H)
        r_vim = load2d("r_vim", 1, H, BF)
        tt = load_act("tT", "tT")
        w_qtx = load("w_qtx", 4, H)
        b_qtx = load2d("b_qtx", 128, 4, F32)
        w_ktx = load("w_ktx", 4, H)
        b_ktx = load2d("b_ktx", 128, 4, F32)
        w_vtx = load("w_vtx", 4, H)
        r_vtx = load2d("r_vtx", 1, H, BF)
        w_oim = load("w_oim", 4, H)
        b_oim = load2d("b_oim", 128, 4, F32)
        w_otx = load("w_otx", 4, H)
        b_otx = load2d("b_otx", 128, 4, F32)
        w_cat = load("w_cat", 8, H)
        b_cat = load2d("b_cat", 128, 4, F32)
        w_ip = load("w_ip", 4, 3 * H)
        b_ipqk = load2d("b_ipqk", 128, 8, F32)
        w_op = load("w_op", 4, H)
        r_op = load2d("r_op", 1, H, BF)

        ones_row = const.tile([1, 128], BF, tag="ones_row")
        nc.vector.memset(ones_row, 1.0)

        # PE warmup: dep-free matmuls that run under the initial input DMAs,
        # so the HAM clock gate reaches K=8/8 (2.4 GHz) before real work.
        warm_mv = const.tile([1, 512], BF, tag="warm_mv")
        nc.vector.memset(warm_mv, 1.0)

        def warm(n, pool, tag):
            wps = pool.tile([128, 512], F32, tag=tag, name="wps")
            for _ in range(n):
                nc.tensor.matmul(wps, ones_row, warm_mv, start=True, stop=True,
                                 skip_group_check=True)

        warm(24, pctx, "ctx")

        # ---- activation tiles ----
        q_im = acts.tile([128, 4, L], BF, tag="q_im")
        k_im = acts.tile([128, 4, L], BF, tag="k_im")
        v_im = acts.tile([128, 8, 8, 65], BF, tag="v_im")
        nc.vector.memset(v_im, 1.0)
        q_tx = acts.tile([128, 4, L], BF, tag="q_tx")
        k_tx = acts.tile([128, 4, L], BF, tag="k_tx")
        v_tx = acts.tile([128, 8, 8, 65], BF, tag="v_tx")
        nc.vector.memset(v_tx, 1.0)
        out_t = opool.tile([128, 4, L], BF, tag="out")

        # ---- filler chunk constructors ----
        # ev="v": psum eviction on VectorE (default, for chunks inside
        # ACT-bound attention windows). ev="s": eviction on ScalarE — used in
        # the serial A4->A5 chain and the tail, where the Scalar engine is
        # idle anyway and VectorE would otherwise pace the chain.
        Ident = mybir.ActivationFunctionType.Identity

        def pt(dst, m, src, w, w_off, bias, bias_off, n0=0, n1=2, ev="v"):
            """feature-major linear, one m-chunk over token blocks [n0,n1)."""

            def emit():
                ps = pmm.tile([128, (n1 - n0) * 512], F32, tag="mm", name="pst")
                for n in range(n0, n1):
                    c0 = (n - n0) * 512
                    for k in range(4):
                        nc.tensor.matmul(
                            ps[:, c0 : c0 + 512],
                            w[:, k, w_off + m * 128 : w_off + (m + 1) * 128],
                            src[:, k, n * 512 : (n + 1) * 512],
                            start=(k == 0),
                            stop=(k == 3),
                        )
                o = dst[:, m, n0 * 512 : n1 * 512]
                b = bias[:, bias_off + m : bias_off + m + 1]
                if ev == "s":
                    nc.scalar.activation(o, ps, Ident, bias=b)
                else:
                    nc.vector.tensor_scalar_add(o, ps, b)
            return emit

        def pn(vdst, src, w, w_off, brow, lc2, ev="v"):
            """natural-orientation linear into the ones-augmented V layout,
            one lc2 chunk (token blocks 2*lc2, 2*lc2+1, all heads)."""

            def emit():
                ps = pmm.tile([128, 1024], F32, tag="mm", name="psn")
                for h in range(2):
                    lc = lc2 * 2 + h
                    for k in range(4):
                        nc.tensor.matmul(
                            ps[:, h * 512 : (h + 1) * 512],
                            src[:, k, lc * 128 : (lc + 1) * 128],
                            w[:, k, w_off : w_off + 512],
                            start=(k == 0),
                            stop=(brow is None and k == 3),
                            skip_group_check=True,
                        )
                    if brow is not None:
                        nc.tensor.matmul(
                            ps[:, h * 512 : (h + 1) * 512],
                            ones_row, brow, start=False, stop=True,
                            skip_group_check=True,
                        )
                o = vdst[:, lc2 * 2 : lc2 * 2 + 2, :, 0:64]
                i = ps.rearrange("p (a b) -> p a b", a=2)
                if ev == "s":
                    nc.scalar.copy(out=o, in_=i)
                else:
                    nc.vector.tensor_copy(out=o, in_=i)
            return emit

        def catc(cat_a, cat_b, m, n, ev="v"):
            """cat linear, one (m-chunk, token-block) pair."""

            def emit():
                ps = pmm.tile([128, 512], F32, tag="mm", name="psc")
                for k in range(8):
                    srck = cat_a if k < 4 else cat_b
                    nc.tensor.matmul(
                        ps,
                        w_cat[:, k, m * 128 : (m + 1) * 128],
                        srck[:, k % 4, n * 512 : (n + 1) * 512],
                        start=(k == 0),
                        stop=(k == 7),
                    )
                o = out_t[:, m, n * 512 : (n + 1) * 512]
                b = b_cat[:, m : m + 1]
                if ev == "s":
                    nc.scalar.activation(o, ps, Ident, bias=b)
                else:
                    nc.vector.tensor_scalar_add(o, ps, b)
            return emit

        # ---- attention with split lag normalization + filler windows ----
        # Normalization of pair p is split: the A-part (denominator scaled
        # copy, fast reciprocal, DMA broadcast through DRAM) is emitted at
        # the START of window p+1 — so it heads the Vector FIFO right when
        # cps(p) is complete and its 2-hop broadcast DMA has a full window
        # to land — and the B-part (the psum-evicting multiplies) at the
        # start of window p+2.  No DVE op ever waits on an in-flight DMA,
        # so filler evictions never delay the psum buffer rotation that
        # feeds the Scalar engine's exp stream.
        pendA = []
        pendB = []

        def flushA_all():
            while pendA:
                pendA.pop(0)()

        def flush_all():
            flushA_all()
            while pendB:
                pendB.pop(0)()

        def attention(qT, kT, vN, s_dst, first, scale, fills=None):
            """One multi-head attention; accumulates normalized ctx' into s_dst.

            fills[(ih,p)]: filler chunks emitted inside that window, after the
            scores+exp stream and the lag-2 normalize-B, before PV."""
            for ih in range(2):
                i0 = ih * 512
                for p in range(4):
                    ex = expool.tile([128, 8, 1024], BF, tag="exp", name="ex")
                    for jt in range(8):
                        ps = pmm.tile([128, 1024], F32, tag="mm", name="scps")
                        for hh in range(2):
                            nc.tensor.matmul(
                                ps[:, hh * 512 : (hh + 1) * 512],
                                kT[hh * 64 : (hh + 1) * 64, p, jt * 128 : (jt + 1) * 128],
                                qT[hh * 64 : (hh + 1) * 64, p, i0 : i0 + 512],
                                start=True,
                                stop=True,
                                tile_position=(hh * 64, 0),
                            )
                        nc.scalar.activation(ex[:, jt, :], ps, Exp)
                    if pendA:
                        pendA.pop(0)()
                    if len(pendB) >= 2:
                        pendB.pop(0)()
                    for f in (fills or {}).get((ih, p), ()):
                        f()
                    cps = pctx.tile([128, 1024], F32, tag="ctx", name="cps")
                    for jt in range(8):
                        for hh in range(2):
                            nc.tensor.matmul(
                                cps[0:65, hh * 512 : (hh + 1) * 512],
                                vN[:, jt, p * 2 + hh, :],
                                ex[:, jt, hh * 512 : (hh + 1) * 512],
                                start=(jt == 0),
                                stop=(jt == 7),
                            )
                    st = {}

                    def normalizeA(cps=cps, scale=scale, st=st):
                        # scaled copy of both denominator rows to SBUF (the
                        # custom recip op's fp32 bit-trick seed reads garbage
                        # from PSUM directly); scale=2 folds the reference's
                        # (ctx_a + ctx_b) * 0.5 averaging into 1/(2*den)
                        den = small.tile([1, 1024], F32, tag="den", name="den")
                        nc.vector.tensor_scalar_mul(den, cps[64:65, :], scale)
                        rc = small.tile([1, 1024], BF, tag="rc", name="rc")
                        cdve = RECIP_APPROX_FAST_CONSTS
                        nc.vector._custom_dve(
                            RECIPROCAL_APPROX_FAST, out=rc, in0=den,
                            s0=cdve["s0"], s1=cdve["s1"], imm2=cdve["imm2"],
                        )
                        # partition-broadcast of the recips via one DMA
                        # through a DRAM scratch row (SBUF APs forbid
                        # stride-0 partitions; DRAM APs allow it)
                        dr = dscr.tile([1, 1024], BF, tag="dr", name="dr")
                        nc.sync.dma_start(out=dr, in_=rc)
                        bcs = small.tile([128, 512], BF, tag="bcs", name="bcs")
                        bsrc = bass.AP(tensor=dr.tensor, offset=dr.offset,
                                       ap=[[512, 2], [0, 64], [1, 512]])
                        nc.sync.dma_start(out=bcs, in_=bsrc)
                        st["bcs"] = bcs

                    def normalizeB(cps=cps, p=p, i0=i0, first=first,
                                   s_dst=s_dst, st=st):
                        bcs = st["bcs"]
                        o = s_dst[:, p, i0 : i0 + 512]
                        if first:
                            nc.vector.tensor_mul(o[0:64, :], cps[0:64, 0:512], bcs[0:64, :])
                            nc.vector.tensor_mul(o[64:128, :], cps[0:64, 512:1024], bcs[64:128, :])
                        else:
                            tmp = small.tile([128, 512], BF, tag="tmp", name="tmp")
                            nc.vector.tensor_mul(tmp[0:64, :], cps[0:64, 0:512], bcs[0:64, :])
                            nc.vector.tensor_mul(tmp[64:128, :], cps[0:64, 512:1024], bcs[64:128, :])
                            nc.vector.tensor_add(o, o, tmp)

                    pendA.append(normalizeA)
                    pendB.append(normalizeB)

        # ---- the network ----
        s_img = spool.tile([128, 4, L], BF, tag="s", name="s_img")

        # head: just enough for A1's first two windows
        pt(q_im, 0, xt, w_qim, 0, b_qim, 0)()
        pt(k_im, 0, xt, w_kim, 0, b_kim, 0)()
        pt(q_im, 1, xt, w_qim, 0, b_qim, 0)()
        pt(k_im, 1, xt, w_kim, 0, b_kim, 0)()

        attention(  # A1: ctx_img
            q_im, k_im, v_im, s_img, True, 2.0,
            fills={
                (0, 0): [pn(v_im, xt, w_vim, 0, r_vim, lc2) for lc2 in range(4)],
                (0, 1): [pt(q_im, 2, xt, w_qim, 0, b_qim, 0),
                         pt(k_im, 2, xt, w_kim, 0, b_kim, 0),
                         pt(q_im, 3, xt, w_qim, 0, b_qim, 0),
                         pt(k_im, 3, xt, w_kim, 0, b_kim, 0)],
                (0, 2): [pt(q_tx, 0, tt, w_qtx, 0, b_qtx, 0),
                         pt(k_tx, 0, tt, w_ktx, 0, b_ktx, 0)],
                (0, 3): [pt(q_tx, 1, tt, w_qtx, 0, b_qtx, 0),
                         pt(k_tx, 1, tt, w_ktx, 0, b_ktx, 0)],
                (1, 0): [pt(q_tx, 2, tt, w_qtx, 0, b_qtx, 0),
                         pt(k_tx, 2, tt, w_ktx, 0, b_ktx, 0)],
                (1, 1): [pt(q_tx, 3, tt, w_qtx, 0, b_qtx, 0),
                         pt(k_tx, 3, tt, w_ktx, 0, b_ktx, 0)],
                (1, 2): [pn(v_tx, tt, w_vtx, 0, r_vtx, 0),
                         pn(v_tx, tt, w_vtx, 0, r_vtx, 1)],
                (1, 3): [pn(v_tx, tt, w_vtx, 0, r_vtx, 2),
                         pn(v_tx, tt, w_vtx, 0, r_vtx, 3)],
            },
        )

        attention(q_im, k_tx, v_tx, s_img, False, 2.0)  # A2: ctx_it

        s_txt = spool.tile([128, 4, L], BF, tag="s", name="s_txt")
        cat_a = acts.tile([128, 4, L], BF, tag="xT", name="cat_a")

        attention(  # A3: ctx_txt  (out_img rides its windows)
            q_tx, k_tx, v_tx, s_txt, True, 2.0,
            fills={
                # s_img is complete once A2 (1,3)'s normalize-B pops at (0,1)
                (0, 2): [pt(cat_a, 0, s_img, w_oim, 0, b_oim, 0)],
                (0, 3): [pt(cat_a, 1, s_img, w_oim, 0, b_oim, 0)],
                (1, 0): [pt(cat_a, 2, s_img, w_oim, 0, b_oim, 0)],
                (1, 1): [pt(cat_a, 3, s_img, w_oim, 0, b_oim, 0)],
            },
        )

        cat_b = acts.tile([128, 4, L], BF, tag="tT", name="cat_b")
        # k_pl reuses q_tx's buffer: its first write is emitted in A4's
        # (1,3) window fills, after A4's last q_tx score read.
        k_pl = acts.tile([128, 4, L], BF, tag="q_tx", name="k_pl")

        attention(  # A4: ctx_ti  (token-half-0 tail chain rides ih1)
            q_tx, k_im, v_im, s_txt, False, 2.0,
            fills={
                # s_txt half-0 is complete once A4 (0,3)'s B pops at (1,1)
                (1, 1): [pt(cat_b, m, s_txt, w_otx, 0, b_otx, 0, n0=0, n1=1)
                         for m in range(4)],
                (1, 2): [catc(cat_a, cat_b, 0, 0), catc(cat_a, cat_b, 1, 0),
                         catc(cat_a, cat_b, 2, 0)],
                (1, 3): [catc(cat_a, cat_b, 3, 0),
                         pt(k_pl, 0, out_t, w_ip, 512, b_ipqk, 4, n0=0, n1=1),
                         pt(k_pl, 1, out_t, w_ip, 512, b_ipqk, 4, n0=0, n1=1)],
            },
        )

        # serial remainder of the tail chain (token half 1 + v_pl); the PE
        # chunks with met deps go first so they run while the trailing
        # normalize-Bs drain on the Vector engine.
        q_pl = acts.tile([128, 4, L], BF, tag="q_im", name="q_pl")
        v_pl = acts.tile([128, 8, 8, 65], BF, tag="v_im", name="v_pl")
        nc.vector.memset(v_pl, 1.0)
        pt(k_pl, 2, out_t, w_ip, 512, b_ipqk, 4, n0=0, n1=1, ev="s")()
        pt(k_pl, 3, out_t, w_ip, 512, b_ipqk, 4, n0=0, n1=1, ev="s")()
        pn(v_pl, out_t, w_ip, 1024, None, 0, ev="s")()
        pn(v_pl, out_t, w_ip, 1024, None, 1, ev="s")()
        flush_all()
        for m in range(4):
            pt(cat_b, m, s_txt, w_otx, 0, b_otx, 0, n0=1, n1=2, ev="s")()
        for m in range(4):
            catc(cat_a, cat_b, m, 1, ev="s")()
        pt(k_pl, 0, out_t, w_ip, 512, b_ipqk, 4, n0=1, n1=2, ev="s")()
        pt(q_pl, 0, out_t, w_ip, 0, b_ipqk, 0, n0=0, n1=1, ev="s")()

        ctx_p = spool.tile([128, 4, L], BF, tag="s", name="ctx_p")

        def op_chunk(lc, ev="v"):
            # out_proj (natural orientation) + bias, streamed to DRAM
            def emit():
                ps = pmm.tile([128, 512], F32, tag="mm", name="pso")
                for k in range(4):
                    nc.tensor.matmul(
                        ps, ctx_p[:, k, lc * 128 : (lc + 1) * 128], w_op[:, k, :],
                        start=(k == 0), stop=False, skip_group_check=True,
                    )
                nc.tensor.matmul(ps, ones_row, r_op, start=False, stop=True,
                                 skip_group_check=True)
                res = small.tile([128, 512], F32, tag="res", name="res")
                if ev == "s":
                    nc.scalar.copy(out=res, in_=ps)
                else:
                    nc.vector.tensor_copy(out=res, in_=ps)
                nc.sync.dma_start(out=d["out"][lc * 128 : (lc + 1) * 128, :], in_=res)
            return emit

        attention(  # A5: pooling attention (in_proj q + out_proj ride it)
            q_pl, k_pl, v_pl, ctx_p, True, 1.0,
            fills={
                # each in_proj chunk lands one window before the scores that
                # read it (fills follow that window's own scores)
                (0, 0): [pn(v_pl, out_t, w_ip, 1024, None, 2),
                         pn(v_pl, out_t, w_ip, 1024, None, 3),
                         pt(k_pl, 1, out_t, w_ip, 512, b_ipqk, 4, n0=1, n1=2),
                         pt(q_pl, 1, out_t, w_ip, 0, b_ipqk, 0, n0=0, n1=1)],
                (0, 1): [pt(k_pl, 2, out_t, w_ip, 512, b_ipqk, 4, n0=1, n1=2),
                         pt(q_pl, 2, out_t, w_ip, 0, b_ipqk, 0, n0=0, n1=1)],
                (0, 2): [pt(k_pl, 3, out_t, w_ip, 512, b_ipqk, 4, n0=1, n1=2),
                         pt(q_pl, 3, out_t, w_ip, 0, b_ipqk, 0, n0=0, n1=1)],
                (0, 3): [pt(q_pl, 0, out_t, w_ip, 0, b_ipqk, 0, n0=1, n1=2)],
                (1, 0): [pt(q_pl, 1, out_t, w_ip, 0, b_ipqk, 0, n0=1, n1=2)],
                # ctx_p ih0 is complete once A5 (0,3)'s B pops at (1,1)
                (1, 1): [pt(q_pl, 2, out_t, w_ip, 0, b_ipqk, 0, n0=1, n1=2),
                         op_chunk(0)],
                (1, 2): [pt(q_pl, 3, out_t, w_ip, 0, b_ipqk, 0, n0=1, n1=2),
                         op_chunk(1), op_chunk(2)],
                (1, 3): [op_chunk(3)],
            },
        )
        # tail: issue the last pair's broadcast DMA first, keep the PE warm
        # under it with dep-free matmuls, then evict and stream out.
        flushA_all()
        warm(10, pmm, "mm")
        flush_all()
        for lc in range(4, 8):
            op_chunk(lc, ev="s")()


_PROGRAM = None


def _build_program():
    global _PROGRAM
    if _PROGRAM is not None:
        return _PROGRAM
    nc = bacc.Bacc("TRN2", target_bir_lowering=False, debug=False)
    d = {}

    def din(name, shape, dt):
        d[name] = nc.dram_tensor(name, list(shape), dt, kind="ExternalInput").ap()

    din("xT", (H, L), BF)
    din("tT", (H, L), BF)
    for n in ("w_qim", "w_kim", "w_vim", "w_qtx", "w_ktx", "w_vtx", "w_oim", "w_otx"):
        din(n, (H, H), BF)
    din("w_cat", (2 * H, H), BF)
    din("w_ip", (H, 3 * H), BF)
    din("w_op", (H, H), BF)
    for n in ("b_qim", "b_kim", "b_qtx", "b_ktx", "b_oim", "b_otx", "b_cat"):
        din(n, (128, 4), F32)
    din("b_ipqk", (128, 8), F32)
    for n in ("r_vim", "r_vtx", "r_op"):
        din(n, (1, H), BF)
    d["out"] = nc.dram_tensor("out", [L, H], F32, kind="ExternalOutput").ap()

    with tile.TileContext(nc) as tc:
        _emit(tc, d)
    nc.compile()
    _PROGRAM = nc
    return nc


def _host_prep(inputs):
    f = lambda x: np.asarray(x, np.float32)

    def wT(w, scale=None):
        w = f(w)
        if scale is not None:
            w = w * scale
        return np.ascontiguousarray(w.T).astype(bf16)

    def bcol(b, scale=None):
        b = f(b)
        if scale is not None:
            b = b * scale
        return np.ascontiguousarray(b.reshape(-1, 128).T.astype(np.float32))

    def brow(b):
        return f(b).astype(bf16).reshape(1, -1)

    s = 1.0 / np.sqrt(HD)
    ipw = f(inputs["in_proj_w"]).copy()
    ipw[0:H] *= s
    ipb = f(inputs["in_proj_b"]).copy()
    ipb[0:H] *= s

    shared = {
        "w_qim": wT(inputs["w_q_img"], s),
        "w_kim": wT(inputs["w_k_img"]),
        "w_vim": wT(inputs["w_v_img"]),
        "w_qtx": wT(inputs["w_q_txt"], s),
        "w_ktx": wT(inputs["w_k_txt"]),
        "w_vtx": wT(inputs["w_v_txt"]),
        "w_oim": wT(inputs["w_out_img"]),
        "w_otx": wT(inputs["w_out_txt"]),
        "w_cat": wT(inputs["w_cat"]),
        "w_ip": wT(ipw),
        "w_op": wT(inputs["out_proj_w"]),
        "b_qim": bcol(inputs["b_q_img"], s),
        "b_kim": bcol(inputs["b_k_img"]),
        "b_qtx": bcol(inputs["b_q_txt"], s),
        "b_ktx": bcol(inputs["b_k_txt"]),
        "b_oim": bcol(inputs["b_out_img"]),
        "b_otx": bcol(inputs["b_out_txt"]),
        "b_cat": bcol(inputs["b_cat"]),
        "b_ipqk": bcol(ipb[0 : 2 * H]),
        "r_vim": brow(inputs["b_v_img"]),
        "r_vtx": brow(inputs["b_v_txt"]),
        "r_op": brow(inputs["out_proj_b"]),
    }
    hs = f(inputs["hidden_states"])
    tx = f(inputs["text"])
    in_maps = []
    for c in range(N_CORES):
        m = dict(shared)
        m["xT"] = np.ascontiguousarray(hs[c].T).astype(bf16)
        m["tT"] = np.ascontiguousarray(tx[c].T).astype(bf16)
        in_maps.append(m)
    return in_maps


def kernel(**inputs):
    nc = _build_program()
    in_maps = _host_prep(inputs)
    res = run_bass_kernel_spmd(nc, in_maps, core_ids=list(range(N_CORES)))
    out = np.stack([res.results[c]["out"] for c in range(N_CORES)])
    return out.astype(np.float32)


# revision 26
# speedup vs baseline: 1.1397x; 1.1397x over previous
"""Trainium2 Bass kernel for the dual-modality dense transformer block.

Problem (hardcoded shapes): B=8, L=1024, H=512, NH=8, HD=64.
  - 6 linear projections (q/k/v for img and txt streams)
  - 4 full attentions: (q_img,KV_img), (q_txt,KV_txt), (q_img,KV_txt), (q_txt,KV_img)
  - out_img/out_txt linears on the averaged contexts, concat + cat linear
  - attention pooling (nn.MultiheadAttention-style) + out_proj

Sharding: pure data-parallel over batch B=8 across the 8 NeuronCores.

Device algorithm (per core, one batch element): identical math to the
previous version (feature-major activations, transposed scores, ones-
augmented V for free softmax denominators, lag-1 DVE normalization with a
DMA-broadcast reciprocal).  What changed is the EMISSION SCHEDULE: the
Scalar engine's exp stream (320 activations x ~1us = the kernel's floor)
is kept saturated by interleaving every projection as small "filler"
chunks inside the attention windows instead of as serial blocks between
attentions:
  - head: only q_img/k_img chunk 0+1 before attention 1 starts
  - v_img is emitted between A1's first scores and first PV group
  - remaining q/k chunks and all txt projections ride A1's windows
  - out_img rides A3, out_txt/cat/in_proj(k) token-half 0 rides A4-ih1
  - in_proj(q) and the out_proj epilogue ride A5's windows
This removes the ~110us of Scalar-engine idle (phase-boundary
serialization) the previous schedule had.
"""

import numpy as np
import ml_dtypes

import concourse.bass as bass
import concourse.tile as tile
from concourse import bacc, mybir
from concourse.bass_utils import run_bass_kernel_spmd
from concourse.dve_ops import RECIP_APPROX_FAST_CONSTS, RECIPROCAL_APPROX_FAST

B, L, H, NH, HD = 8, 1024, 512, 8, 64
BF = mybir.dt.bfloat16
F32 = mybir.dt.float32
Exp = mybir.ActivationFunctionType.Exp
bf16 = ml_dtypes.bfloat16

N_CORES = 8


def _emit(tc, d):
    nc = tc.nc
    import contextlib

    ctx = contextlib.ExitStack()
    with ctx:
        const = ctx.enter_context(tc.tile_pool(name="const", bufs=1))
        acts = ctx.enter_context(tc.tile_pool(name="acts", bufs=1))
        spool = ctx.enter_context(tc.tile_pool(name="spool", bufs=2))
        opool = ctx.enter_context(tc.tile_pool(name="opool", bufs=1))
        expool = ctx.enter_context(tc.tile_pool(name="expool", bufs=2))
        small = ctx.enter_context(tc.tile_pool(name="small", bufs=2))
        dscr = ctx.enter_context(tc.tile_pool(name="dscr", bufs=4, space="DRAM"))
        pmm = ctx.enter_context(tc.tile_pool(name="pmm", bufs=2, space="PSUM"))
        pctx = ctx.enter_context(tc.tile_pool(name="pctx", bufs=2, space="PSUM"))

        # ---- constants / inputs into SBUF (critical-path order) ----
        def load(name, p_chunks, free, dt=BF):
            t = const.tile([128, p_chunks, free], dt, tag=name, name=name)
            src_r = d[name].rearrange("(c p) n -> p c n", p=128)
            for c in range(p_chunks):
                nc.sync.dma_start(out=t[:, c, :], in_=src_r[:, c, :])
            return t

        def load_act(name, tag):
            t = acts.tile([128, 4, L], BF, tag=tag, name=name)
            src_r = d[name].rearrange("(c p) n -> p c n", p=128)
            for c in range(4):
                nc.sync.dma_start(out=t[:, c, :], in_=src_r[:, c, :])
            return t

        def load2d(name, p, free, dt):
            t = const.tile([p, free], dt, tag=name, name=name)
            nc.sync.dma_start(out=t, in_=d[name])
            return t

        xt = load_act("xT", "xT")
        w_qim = load("w_qim", 4, H)
        b_qim = load2d("b_qim", 128, 4, F32)
        w_kim = load("w_kim", 4, H)
        b_kim = load2d("b_kim", 128, 4, F32)
        w_vim = load("w_vim", 4, H)
        r_vim = load2d("r_vim", 1, H, BF)
        tt = load_act("tT", "tT")
        w_qtx = load("w_qtx", 4, H)
        b_qtx = load2d("b_qtx", 128, 4, F32)
        w_ktx = load("w_ktx", 4, H)
        b_ktx = load2d("b_ktx", 128, 4, F32)
        w_vtx = load("w_vtx", 4, H)
        r_vtx = load2d("r_vtx", 1, H, BF)
        w_oim = load("w_oim", 4, H)
        b_oim = load2d("b_oim", 128, 4, F32)
        w_otx = load("w_otx", 4, H)
        b_otx = load2d("b_otx", 128, 4, F32)
        w_cat = load("w_cat", 8, H)
        b_cat = load2d("b_cat", 128, 4, F32)
        w_ip = load("w_ip", 4, 3 * H)
        b_ipqk = load2d("b_ipqk", 128, 8, F32)
        w_op = load("w_op", 4, H)
        r_op = load2d("r_op", 1, H, BF)

        ones_row = const.tile([1, 128], BF, tag="ones_row")
        nc.vector.memset(ones_row, 1.0)

        # PE warmup: dep-free matmuls that run under the initial input DMAs,
        # so the HAM clock gate reaches K=8/8 (2.4 GHz) before real work.
        warm_mv = const.tile([1, 512], BF, tag="warm_mv")
        nc.vector.memset(warm_mv, 1.0)

        def warm(n, pool, tag):
            wps = pool.tile([128, 512], F32, tag=tag, name="wps")
            for _ in range(n):
                nc.tensor.matmul(wps, ones_row, warm_mv, start=True, stop=True,
                                 skip_group_check=True)

        warm(24, pctx, "ctx")
        # GPSIMD ucode warm: pay the broadcast library's IRAM load under the
        # initial input DMAs, not inside the first attention window.
        gwarm = const.tile([64, 512], BF, tag="gwarm")
        nc.gpsimd.partition_broadcast(gwarm, warm_mv)

        # ---- activation tiles ----
        q_im = acts.tile([128, 4, L], BF, tag="q_im")
        k_im = acts.tile([128, 4, L], BF, tag="k_im")
        v_im = acts.tile([128, 8, 8, 65], BF, tag="v_im")
        nc.vector.memset(v_im, 1.0)
        q_tx = acts.tile([128, 4, L], BF, tag="q_tx")
        k_tx = acts.tile([128, 4, L], BF, tag="k_tx")
        v_tx = acts.tile([128, 8, 8, 65], BF, tag="v_tx")
        nc.vector.memset(v_tx, 1.0)
        out_t = opool.tile([128, 4, L], BF, tag="out")

        # ---- filler chunk constructors ----
        # ev="v": psum eviction on VectorE (default, for chunks inside
        # ACT-bound attention windows). ev="s": eviction on ScalarE — used in
        # the serial A4->A5 chain and the tail, where the Scalar engine is
        # idle anyway and VectorE would otherwise pace the chain.
        Ident = mybir.ActivationFunctionType.Identity

        def pt(dst, m, src, w, w_off, bias, bias_off, n0=0, n1=2, ev="v"):
            """feature-major linear, one m-chunk over token blocks [n0,n1)."""

            def emit():
                ps = pmm.tile([128, (n1 - n0) * 512], F32, tag="mm", name="pst")
                for n in range(n0, n1):
                    c0 = (n - n0) * 512
                    for k in range(4):
                        nc.tensor.matmul(
                            ps[:, c0 : c0 + 512],
                            w[:, k, w_off + m * 128 : w_off + (m + 1) * 128],
                            src[:, k, n * 512 : (n + 1) * 512],
                            start=(k == 0),
                            stop=(k == 3),
                        )
                o = dst[:, m, n0 * 512 : n1 * 512]
                b = bias[:, bias_off + m : bias_off + m + 1]
                if ev == "s":
                    nc.scalar.activation(o, ps, Ident, bias=b)
                else:
                    nc.vector.tensor_scalar_add(o, ps, b)
            return emit

        def pn(vdst, src, w, w_off, brow, lc2, ev="v"):
            """natural-orientation linear into the ones-augmented V layout,
            one lc2 chunk (token blocks 2*lc2, 2*lc2+1, all heads)."""

            def emit():
                ps = pmm.tile([128, 1024], F32, tag="mm", name="psn")
                for h in range(2):
                    lc = lc2 * 2 + h
                    for k in range(4):
                        nc.tensor.matmul(
                            ps[:, h * 512 : (h + 1) * 512],
                            src[:, k, lc * 128 : (lc + 1) * 128],
                            w[:, k, w_off : w_off + 512],
                            start=(k == 0),
                            stop=(brow is None and k == 3),
                            skip_group_check=True,
                        )
                    if brow is not None:
                        nc.tensor.matmul(
                            ps[:, h * 512 : (h + 1) * 512],
                            ones_row, brow, start=False, stop=True,
                            skip_group_check=True,
                        )
                o = vdst[:, lc2 * 2 : lc2 * 2 + 2, :, 0:64]
                i = ps.rearrange("p (a b) -> p a b", a=2)
                if ev == "s":
                    nc.scalar.copy(out=o, in_=i)
                else:
                    nc.vector.tensor_copy(out=o, in_=i)
            return emit

        def catc(cat_a, cat_b, m, n, ev="v"):
            """cat linear, one (m-chunk, token-block) pair."""

            def emit():
                ps = pmm.tile([128, 512], F32, tag="mm", name="psc")
                for k in range(8):
                    srck = cat_a if k < 4 else cat_b
                    nc.tensor.matmul(
                        ps,
                        w_cat[:, k, m * 128 : (m + 1) * 128],
                        srck[:, k % 4, n * 512 : (n + 1) * 512],
                        start=(k == 0),
                        stop=(k == 7),
                    )
                o = out_t[:, m, n * 512 : (n + 1) * 512]
                b = b_cat[:, m : m + 1]
                if ev == "s":
                    nc.scalar.activation(o, ps, Ident, bias=b)
                else:
                    nc.vector.tensor_scalar_add(o, ps, b)
            return emit

        # ---- attention with split lag-2 normalization + filler windows ----
        # Normalization of pair p is split: the A-part (denominator scaled
        # copy, fast reciprocal, GPSIMD partition-broadcast of the recips)
        # is emitted right after pair p's PV; the B-part (the psum-evicting
        # multiplies) at the start of window p+2, by which point the
        # broadcast — running on the otherwise-idle GPSIMD engine — has long
        # finished.  No Vector-engine op ever waits on another engine, so
        # filler evictions never delay the psum buffer rotation that feeds
        # the Scalar engine's exp stream.
        pendB = []

        def flush_all():
            while pendB:
                pendB.pop(0)()

        def attention(qT, kT, vN, s_dst, first, scale, fills=None):
            """One multi-head attention; accumulates normalized ctx' into s_dst.

            fills[(ih,p)]: filler chunks emitted inside that window, after the
            scores+exp stream and the lag-2 normalize-B, before PV."""
            for ih in range(2):
                i0 = ih * 512
                for p in range(4):
                    ex = expool.tile([128, 8, 1024], BF, tag="exp", name="ex")
                    for jt in range(8):
                        ps = pmm.tile([128, 1024], F32, tag="mm", name="scps")
                        for hh in range(2):
                            nc.tensor.matmul(
                                ps[:, hh * 512 : (hh + 1) * 512],
                                kT[hh * 64 : (hh + 1) * 64, p, jt * 128 : (jt + 1) * 128],
                                qT[hh * 64 : (hh + 1) * 64, p, i0 : i0 + 512],
                                start=True,
                                stop=True,
                                tile_position=(hh * 64, 0),
                            )
                        nc.scalar.activation(ex[:, jt, :], ps, Exp)
                    if len(pendB) >= 2:
                        pendB.pop(0)()
                    for f in (fills or {}).get((ih, p), ()):
                        f()
                    cps = pctx.tile([128, 1024], F32, tag="ctx", name="cps")
                    for jt in range(8):
                        for hh in range(2):
                            nc.tensor.matmul(
                                cps[0:65, hh * 512 : (hh + 1) * 512],
                                vN[:, jt, p * 2 + hh, :],
                                ex[:, jt, hh * 512 : (hh + 1) * 512],
                                start=(jt == 0),
                                stop=(jt == 7),
                            )
                    # normalize A-part: scaled copy of both denominator rows
                    # to SBUF (the custom recip op's fp32 bit-trick seed reads
                    # garbage from PSUM directly); scale=2 folds the
                    # reference's (ctx_a + ctx_b) * 0.5 averaging into
                    # 1/(2*den).  The recips are partition-broadcast by the
                    # GPSIMD engine (SBUF stride-0 partition APs are illegal,
                    # and a DMA bounce through DRAM costs ~5us of latency).
                    den = small.tile([1, 1024], F32, tag="den", name="den")
                    nc.vector.tensor_scalar_mul(den, cps[64:65, :], scale)
                    rc = small.tile([1, 1024], BF, tag="rc", name="rc")
                    cdve = RECIP_APPROX_FAST_CONSTS
                    nc.vector._custom_dve(
                        RECIPROCAL_APPROX_FAST, out=rc, in0=den,
                        s0=cdve["s0"], s1=cdve["s1"], imm2=cdve["imm2"],
                    )
                    bcs = small.tile([128, 1024], BF, tag="bcs", name="bcs")
                    nc.gpsimd.partition_broadcast(bcs, rc)

                    def normalizeB(cps=cps, bcs=bcs, p=p, i0=i0, first=first,
                                   s_dst=s_dst):
                        o = s_dst[:, p, i0 : i0 + 512]
                        if first:
                            nc.vector.tensor_mul(o[0:64, :], cps[0:64, 0:512], bcs[0:64, 0:512])
                            nc.vector.tensor_mul(o[64:128, :], cps[0:64, 512:1024], bcs[64:128, 512:1024])
                        else:
                            tmp = small.tile([128, 512], BF, tag="tmp", name="tmp")
                            nc.vector.tensor_mul(tmp[0:64, :], cps[0:64, 0:512], bcs[0:64, 0:512])
                            nc.vector.tensor_mul(tmp[64:128, :], cps[0:64, 512:1024], bcs[64:128, 512:1024])
                            nc.vector.tensor_add(o, o, tmp)

                    pendB.append(normalizeB)

        # ---- the network ----
        s_img = spool.tile([128, 4, L], BF, tag="s", name="s_img")

        # head: just enough for A1's first two windows
        pt(q_im, 0, xt, w_qim, 0, b_qim, 0)()
        pt(k_im, 0, xt, w_kim, 0, b_kim, 0)()
        pt(q_im, 1, xt, w_qim, 0, b_qim, 0)()
        pt(k_im, 1, xt, w_kim, 0, b_kim, 0)()

        attention(  # A1: ctx_img
            q_im, k_im, v_im, s_img, True, 2.0,
            fills={
                (0, 0): [pn(v_im, xt, w_vim, 0, r_vim, lc2) for lc2 in range(4)],
                (0, 1): [pt(q_im, 2, xt, w_qim, 0, b_qim, 0),
                         pt(k_im, 2, xt, w_kim, 0, b_kim, 0),
                         pt(q_im, 3, xt, w_qim, 0, b_qim, 0),
                         pt(k_im, 3, xt, w_kim, 0, b_kim, 0)],
                (0, 2): [pt(q_tx, 0, tt, w_qtx, 0, b_qtx, 0),
                         pt(k_tx, 0, tt, w_ktx, 0, b_ktx, 0)],
                (0, 3): [pt(q_tx, 1, tt, w_qtx, 0, b_qtx, 0),
                         pt(k_tx, 1, tt, w_ktx, 0, b_ktx, 0)],
                (1, 0): [pt(q_tx, 2, tt, w_qtx, 0, b_qtx, 0),
                         pt(k_tx, 2, tt, w_ktx, 0, b_ktx, 0)],
                (1, 1): [pt(q_tx, 3, tt, w_qtx, 0, b_qtx, 0),
                         pt(k_tx, 3, tt, w_ktx, 0, b_ktx, 0)],
                (1, 2): [pn(v_tx, tt, w_vtx, 0, r_vtx, 0),
                         pn(v_tx, tt, w_vtx, 0, r_vtx, 1)],
                (1, 3): [pn(v_tx, tt, w_vtx, 0, r_vtx, 2),
                         pn(v_tx, tt, w_vtx, 0, r_vtx, 3)],
            },
        )

        attention(q_im, k_tx, v_tx, s_img, False, 2.0)  # A2: ctx_it

        s_txt = spool.tile([128, 4, L], BF, tag="s", name="s_txt")
        cat_a = acts.tile([128, 4, L], BF, tag="xT", name="cat_a")

        attention(  # A3: ctx_txt  (out_img rides its windows)
            q_tx, k_tx, v_tx, s_txt, True, 2.0,
            fills={
                # s_img is complete once A2 (1,3)'s normalize-B pops at (0,1)
                (0, 2): [pt(cat_a, 0, s_img, w_oim, 0, b_oim, 0)],
                (0, 3): [pt(cat_a, 1, s_img, w_oim, 0, b_oim, 0)],
                (1, 0): [pt(cat_a, 2, s_img, w_oim, 0, b_oim, 0)],
                (1, 1): [pt(cat_a, 3, s_img, w_oim, 0, b_oim, 0)],
            },
        )

        cat_b = acts.tile([128, 4, L], BF, tag="tT", name="cat_b")
        # k_pl reuses q_tx's buffer: its first write is emitted in A4's
        # (1,3) window fills, after A4's last q_tx score read.
        k_pl = acts.tile([128, 4, L], BF, tag="q_tx", name="k_pl")

        attention(  # A4: ctx_ti  (token-half-0 tail chain rides ih1)
            q_tx, k_im, v_im, s_txt, False, 2.0,
            fills={
                # s_txt half-0 is complete once A4 (0,3)'s B pops at (1,1)
                (1, 1): [pt(cat_b, m, s_txt, w_otx, 0, b_otx, 0, n0=0, n1=1)
                         for m in range(4)],
                (1, 2): [catc(cat_a, cat_b, 0, 0), catc(cat_a, cat_b, 1, 0),
                         catc(cat_a, cat_b, 2, 0)],
                (1, 3): [catc(cat_a, cat_b, 3, 0),
                         pt(k_pl, 0, out_t, w_ip, 512, b_ipqk, 4, n0=0, n1=1),
                         pt(k_pl, 1, out_t, w_ip, 512, b_ipqk, 4, n0=0, n1=1)],
            },
        )

        # serial remainder of the tail chain (token half 1 + v_pl); the PE
        # chunks with met deps go first so they run while the trailing
        # normalize-Bs drain on the Vector engine.
        q_pl = acts.tile([128, 4, L], BF, tag="q_im", name="q_pl")
        v_pl = acts.tile([128, 8, 8, 65], BF, tag="v_im", name="v_pl")
        nc.vector.memset(v_pl, 1.0)
        pt(k_pl, 2, out_t, w_ip, 512, b_ipqk, 4, n0=0, n1=1, ev="s")()
        pt(k_pl, 3, out_t, w_ip, 512, b_ipqk, 4, n0=0, n1=1, ev="s")()
        pn(v_pl, out_t, w_ip, 1024, None, 0, ev="s")()
        pn(v_pl, out_t, w_ip, 1024, None, 1, ev="s")()
        flush_all()
        for m in range(4):
            pt(cat_b, m, s_txt, w_otx, 0, b_otx, 0, n0=1, n1=2, ev="s")()
        for m in range(4):
            catc(cat_a, cat_b, m, 1, ev="s")()
        pt(k_pl, 0, out_t, w_ip, 512, b_ipqk, 4, n0=1, n1=2, ev="s")()
        pt(q_pl, 0, out_t, w_ip, 0, b_ipqk, 0, n0=0, n1=1, ev="s")()

        ctx_p = spool.tile([128, 4, L], BF, tag="s", name="ctx_p")

        def op_chunk(lc, ev="v"):
            # out_proj (natural orientation) + bias, streamed to DRAM
            def emit():
                ps = pmm.tile([128, 512], F32, tag="mm", name="pso")
                for k in range(4):
                    nc.tensor.matmul(
                        ps, ctx_p[:, k, lc * 128 : (lc + 1) * 128], w_op[:, k, :],
                        start=(k == 0), stop=False, skip_group_check=True,
                    )
                nc.tensor.matmul(ps, ones_row, r_op, start=False, stop=True,
                                 skip_group_check=True)
                res = small.tile([128, 512], F32, tag="res", name="res")
                if ev == "s":
                    nc.scalar.copy(out=res, in_=ps)
                else:
                    nc.vector.tensor_copy(out=res, in_=ps)
                nc.sync.dma_start(out=d["out"][lc * 128 : (lc + 1) * 128, :], in_=res)
            return emit

        attention(  # A5: pooling attention (in_proj q + out_proj ride it)
            q_pl, k_pl, v_pl, ctx_p, True, 1.0,
            fills={
                # each in_proj chunk lands one window before the scores that
                # read it (fills follow that window's own scores)
                (0, 0): [pn(v_pl, out_t, w_ip, 1024, None, 2),
                         pn(v_pl, out_t, w_ip, 1024, None, 3),
                         pt(k_pl, 1, out_t, w_ip, 512, b_ipqk, 4, n0=1, n1=2),
                         pt(q_pl, 1, out_t, w_ip, 0, b_ipqk, 0, n0=0, n1=1)],
                (0, 1): [pt(k_pl, 2, out_t, w_ip, 512, b_ipqk, 4, n0=1, n1=2),
                         pt(q_pl, 2, out_t, w_ip, 0, b_ipqk, 0, n0=0, n1=1)],
                (0, 2): [pt(k_pl, 3, out_t, w_ip, 512, b_ipqk, 4, n0=1, n1=2),
                         pt(q_pl, 3, out_t, w_ip, 0, b_ipqk, 0, n0=0, n1=1)],
                (0, 3): [pt(q_pl, 0, out_t, w_ip, 0, b_ipqk, 0, n0=1, n1=2)],
                (1, 0): [pt(q_pl, 1, out_t, w_ip, 0, b_ipqk, 0, n0=1, n1=2)],
                # ctx_p ih0 is complete once A5 (0,3)'s B pops at (1,1)
                (1, 1): [pt(q_pl, 2, out_t, w_ip, 0, b_ipqk, 0, n0=1, n1=2),
                         op_chunk(0)],
                (1, 2): [pt(q_pl, 3, out_t, w_ip, 0, b_ipqk, 0, n0=1, n1=2),
                         op_chunk(1), op_chunk(2)],
                (1, 3): [op_chunk(3)],
            },
        )
        # tail: keep the PE warm with dep-free matmuls while the last pair's
        # reciprocal broadcast lands, then evict and stream out.
        warm(10, pmm, "mm")
        flush_all()
        for lc in range(4, 8):
            op_chunk(lc, ev="s")()


_PROGRAM = None


def _build_program():
    global _PROGRAM
    if _PROGRAM is not None:
        return _PROGRAM
    nc = bacc.Bacc("TRN2", target_bir_lowering=False, debug=False)
    d = {}

    def din(name, shape, dt):
        d[name] = nc.dram_tensor(name, list(shape), dt, kind="ExternalInput").ap()

    din("xT", (H, L), BF)
    din("tT", (H, L), BF)
    for n in ("w_qim", "w_kim", "w_vim", "w_qtx", "w_ktx", "w_vtx", "w_oim", "w_otx"):
        din(n, (H, H), BF)
    din("w_cat", (2 * H, H), BF)
    din("w_ip", (H, 3 * H), BF)
    din("w_op", (H, H), BF)
    for n in ("b_qim", "b_kim", "b_qtx", "b_ktx", "b_oim", "b_otx", "b_cat"):
        din(n, (128, 4), F32)
    din("b_ipqk", (128, 8), F32)
    for n in ("r_vim", "r_vtx", "r_op"):
        din(n, (1, H), BF)
    d["out"] = nc.dram_tensor("out", [L, H], F32, kind="ExternalOutput").ap()

    with tile.TileContext(nc) as tc:
        _emit(tc, d)
    nc.compile()
    _PROGRAM = nc
    return nc


def _host_prep(inputs):
    f = lambda x: np.asarray(x, np.float32)

    def wT(w, scale=None):
        w = f(w)
        if scale is not None:
            w = w * scale
        return np.ascontiguousarray(w.T).astype(bf16)

    def bcol(b, scale=None):
        b = f(b)
        if scale is not None:
            b = b * scale
        return np.ascontiguousarray(b.reshape(-1, 128).T.astype(np.float32))

    def brow(b):
        return f(b).astype(bf16).reshape(1, -1)

    s = 1.0 / np.sqrt(HD)
    ipw = f(inputs["in_proj_w"]).copy()
    ipw[0:H] *= s
    ipb = f(inputs["in_proj_b"]).copy()
    ipb[0:H] *= s

    shared = {
        "w_qim": wT(inputs["w_q_img"], s),
        "w_kim": wT(inputs["w_k_img"]),
        "w_vim": wT(inputs["w_v_img"]),
        "w_qtx": wT(inputs["w_q_txt"], s),
        "w_ktx": wT(inputs["w_k_txt"]),
        "w_vtx": wT(inputs["w_v_txt"]),
        "w_oim": wT(inputs["w_out_img"]),
        "w_otx": wT(inputs["w_out_txt"]),
        "w_cat": wT(inputs["w_cat"]),
        "w_ip": wT(ipw),
        "w_op": wT(inputs["out_proj_w"]),
        "b_qim": bcol(inputs["b_q_img"], s),
        "b_kim": bcol(inputs["b_k_img"]),
        "b_qtx": bcol(inputs["b_q_txt"], s),
        "b_ktx": bcol(inputs["b_k_txt"]),
        "b_oim": bcol(inputs["b_out_img"]),
        "b_otx": bcol(inputs["b_out_txt"]),
        "b_cat": bcol(inputs["b_cat"]),
        "b_ipqk": bcol(ipb[0 : 2 * H]),
        "r_vim": brow(inputs["b_v_img"]),
        "r_vtx": brow(inputs["b_v_txt"]),
        "r_op": brow(inputs["out_proj_b"]),
    }
    hs = f(inputs["hidden_states"])
    tx = f(inputs["text"])
    in_maps = []
    for c in range(N_CORES):
        m = dict(shared)
        m["xT"] = np.ascontiguousarray(hs[c].T).astype(bf16)
        m["tT"] = np.ascontiguousarray(tx[c].T).astype(bf16)
        in_maps.append(m)
    return in_maps


def kernel(**inputs):
    nc = _build_program()
    in_maps = _host_prep(inputs)
    res = run_bass_kernel_spmd(nc, in_maps, core_ids=list(range(N_CORES)))
    out = np.stack([res.results[c]["out"] for c in range(N_CORES)])
    return out.astype(np.float32)
